# revision 10
# baseline (speedup 1.0000x reference)
"""Trainium2 Bass kernel for nn_AutoregressiveConvLSTM.

Data-parallel over batch: 32 images -> 8 cores x 4 images; on-core the 4
images are split into 2 groups of 2 whose recurrences interleave so the
TensorEngine always has independent work queued (p-state friendly).

All convolutions run as fp8e4m3 DoubleRow band matmuls: a 3x3 SAME conv is
3 vertical-band (tridiagonal 128x128) lhsT matmuls at free-dim offsets
dx in 0..2; DoubleRow contracts TWO (band, plane) pairs per pass at 0.5
cycles/row.  Pairings:
  conv_hh / conv_out : (ci=0, ci=1) per dx           [hpair plane pair]
  conv_ih            : (dx0, dx1-dup) + (dx2, ones)  [bias via ones-plane]
  conv_in            : (dx0, dx1-dup) + (dx2, 0)
The dx pairs need a duplicated plane (x8 duplicated host-side, zi
duplicated by one DVE copy per step) because overlapping stride-1 pair
dims abort at runtime on HW.

Cell math is all-tanh (one ACT table): sigmoid(v) = (tanh(v/2)+1)/2,
h stored doubled (h2 = (tanh(o/2)+1)*tanh(c)), c stored doubled
(ch2 = 2c; ch2' = 0.5*(tf+1)*ch2 + (ti+1)*tg), the g-gate band is
doubled so every gate uses ACT scale 0.5, and the flax forget +1 rides
the ones-plane.  Pointwise ops run in bf16 (DVE 2x mode); z = d*E on the
otherwise-idle Pool engine.

conv_out(t) matmuls are deferred to step t+1's PE stream so the PE never
waits on the cell chain mid-step.
"""

import sys
import numpy as np

for _p in ("/opt/trn_rl_repo", "/root/.axon_site/_ro/trn_rl_repo"):
    if _p not in sys.path:
        sys.path.insert(0, _p)

import ml_dtypes
import concourse.bacc as bacc
import concourse.mybir as mybir
from concourse import bass, tile
from concourse.bass_utils import run_bass_kernel_spmd

F32 = mybir.dt.float32
BF16 = mybir.dt.bfloat16
FP8 = mybir.dt.float8e4
U8 = mybir.dt.uint8
AF = mybir.ActivationFunctionType
ALU = mybir.AluOpType
DR = mybir.MatmulPerfMode.DoubleRow
E4M3 = ml_dtypes.float8_e4m3fn

B, C, H, W = 32, 16, 128, 128
NCORES = 8
BL = B // NCORES          # images per core = 4
BG = 2                    # images per group
GS = (0, 2)               # group start image
WP = W + 2                # padded row width = 130
T = C - 1                 # recurrence steps = 15
LOG2PI = 1.8378770664093453

# pair-band indexing: [H, NPAIR, 2, H] fp8 lhsT pairs
PB_ZI = 0                                     # conv_in (dx0,dx1), (dx2, 0)
PB_IH = lambda co: 2 + co * 5                 # conv_ih (dx0,dx1), (dx2, ones)
PB_HH = lambda co, dx: 2 + co * 5 + 2 + dx    # conv_hh (ci0,ci1) per dx
PB_OUT = lambda co, dx: 42 + co * 3 + dx      # conv_out (ci0,ci1) per dx
NPAIR = 48


def _band(w_col):
    """128x128 vertical-band lhsT: B[h_in, h_out] = w_col[h_in - h_out + 1]."""
    Bm = np.zeros((H, H), np.float32)
    idx = np.arange(H)
    for ky in range(3):
        hh = idx + ky - 1
        m = (hh >= 0) & (hh < H)
        Bm[hh[m], idx[m]] = w_col[ky]
    return Bm


def _build_bands(Win, Wih, Whh, Wout, gb):
    """fp8 pair bands [NPAIR, 2, H, H] + ACT-bias residuals per gate co."""
    bands = np.zeros((NPAIR, 2, H, H), np.float32)
    bands[PB_ZI, 0] = _band(Win[:, 0, 0, 0])
    bands[PB_ZI, 1] = _band(Win[:, 1, 0, 0])
    bands[PB_ZI + 1, 0] = _band(Win[:, 2, 0, 0])

    bias_w = np.zeros(8, np.float64)     # ones-plane weight per gate co
    for co in range(8):
        g = co // 2
        s = 2.0 if g == 1 else 1.0       # g gate: tanh(v+b) = tanh((2v+2b)/2)
        extra = 1.0 if g == 2 else 0.0   # flax forget-gate +1
        bias_w[co] = s * (gb[co] + extra) / H
        bands[PB_IH(co), 0] = s * _band(Wih[:, 0, 0, co])
        bands[PB_IH(co), 1] = s * _band(Wih[:, 1, 0, co])
        bands[PB_IH(co) + 1, 0] = s * _band(Wih[:, 2, 0, co])
        bands[PB_IH(co) + 1, 1] = bias_w[co]     # dense ones-plane weight
        for dx in range(3):
            # h stored as 2h -> x0.5 ; g gate x2
            bands[PB_HH(co, dx), 0] = 0.5 * s * _band(Whh[:, dx, 0, co])
            bands[PB_HH(co, dx), 1] = 0.5 * s * _band(Whh[:, dx, 1, co])
    Wout_y = Wout[:, :, :2, :]
    for co in range(2):
        for dx in range(3):
            bands[PB_OUT(co, dx), 0] = 0.5 * _band(Wout_y[:, dx, 0, co])
            bands[PB_OUT(co, dx), 1] = 0.5 * _band(Wout_y[:, dx, 1, co])

    bands8 = bands.astype(E4M3)
    # ACT-bias residual per co (applied at tanh scale 0.5):
    resid = np.zeros(8, np.float64)
    for co in range(8):
        g = co // 2
        s = 2.0 if g == 1 else 1.0
        extra = 1.0 if g == 2 else 0.0
        want = s * (gb[co] + extra)
        got = float(bands8[PB_IH(co) + 1, 1, 0, 0]) * H
        resid[co] = 0.5 * (want - got)
    return bands8, resid


_CACHED = {}


def _pair_ap(base, stride):
    """Insert a leading free dim [stride, 2] into an AP (DoubleRow pair)."""
    dims = [list(d) for d in base.ap]
    return bass.AP(base.tensor, base.offset, [dims[0], [stride, 2]] + dims[1:])


def _flat(ap):
    """Collapse a contiguous tile AP to a single free dim (for bitcast)."""
    dims = [list(d) for d in ap.ap]
    n = 1
    for d in dims[1:]:
        n *= d[1]
    return bass.AP(ap.tensor, ap.offset, [dims[0], [1, n]])


def _build_program(act_runs):
    """act_runs: per 2-co chunk (i, g, f, o), tuple of (start, n, bias) runs."""
    import os
    TR = int(os.environ.get("KERNEL_T", T))
    nc = bacc.Bacc(None, target_bir_lowering=False)

    xbf_d = nc.dram_tensor("xbf", [H, C * BL * WP], BF16, kind="ExternalInput")
    x8_d = nc.dram_tensor("x8", [H, T * 2 * BL * WP], U8, kind="ExternalInput")
    bands_d = nc.dram_tensor("bands", [H, NPAIR * 2 * H], U8,
                             kind="ExternalInput")
    cols_d = nc.dram_tensor("cols", [H, 16], F32, kind="ExternalInput")
    ones8_d = nc.dram_tensor("ones8", [H, T * BL * WP], U8, kind="ExternalInput")
    out_d = nc.dram_tensor("out", [BL, 1], F32, kind="ExternalOutput")

    ONES8 = float(np.frombuffer(b"\x38" * 4, np.float32)[0])  # fp8 1.0 x4

    with tile.TileContext(nc) as tc:
        with (
            tc.tile_pool(name="const", bufs=1) as cpool,
            tc.tile_pool(name="state", bufs=1) as spool,
            tc.tile_pool(name="work", bufs=2) as wpool,
            tc.tile_pool(name="psum", bufs=2, space=bass.MemorySpace.PSUM) as ppool,
        ):
            xbf = cpool.tile([H, C, BL, WP], BF16, tag="xbf")
            x8 = cpool.tile([H, T, 2, BL, WP], FP8, tag="x8")
            bandsb = cpool.tile([H, NPAIR, 2, H], FP8, tag="bands")
            cols = cpool.tile([H, 16], F32, tag="cols")
            ziall = cpool.tile([H, T, 3, BL, WP], FP8, tag="ziall")

            hpair = spool.tile([H, 2, 2, BG, WP], FP8, tag="hpair")
            ch2 = spool.tile([H, 2, BL, W], BF16, tag="ch2")
            sqcols = spool.tile([H, BL, 16], F32, tag="sqcols")
            lscols = spool.tile([H, BL, 16], F32, tag="lscols")
            ones = spool.tile([H, 1], F32, tag="ones")

            nc.sync.dma_start(bandsb[:].bitcast(U8), bands_d[:])
            nc.sync.dma_start(x8[:].bitcast(U8), x8_d[:])
            nc.sync.dma_start(cols[:], cols_d[:])
            nc.sync.dma_start(xbf[:], xbf_d[:])

            nc.gpsimd.memset(ziall[:, :, 0, :, 0:WP:WP - 1], 0.0)
            nc.sync.dma_start(ziall[:, :, 2].bitcast(U8), ones8_d[:])
            nc.gpsimd.memset(hpair[:], 0.0)
            nc.gpsimd.memset(ch2[:], 0.0)
            nc.gpsimd.memset(sqcols[:], 0.0)
            nc.gpsimd.memset(lscols[:], 0.0)
            nc.gpsimd.memset(ones[:], 1.0)

            def band(i):
                return bandsb[:, i]          # [H, 2, H]

            def x8_pair(t, gs, d0, d1):
                base = x8[:, t, 0, gs:gs + BG, d0:d0 + W]
                return _pair_ap(base, BL * WP + (d1 - d0))

            def zi_pair(t, gs, d0, slot1, d1):
                base = ziall[:, t, 0, gs:gs + BG, d0:d0 + W]
                return _pair_ap(base, slot1 * BL * WP + (d1 - d0))

            def h_pair(g, dx):
                return hpair[:, g, :, :, dx:dx + W]

            # ---- zi conv: x8 -> psum -> (+b_in) fp8 ziall slot0 ----
            def zi_alloc():
                return ppool.tile([H, 2, BG, W], F32, tag="o", bufs=2,
                                  name="zp")

            def emit_zi_mms(t, g, zp):
                gs = GS[g]
                nc.tensor.matmul(zp[:, g], band(PB_ZI), x8_pair(t, gs, 0, 1),
                                 start=True, stop=False, perf_mode=DR)
                nc.tensor.matmul(zp[:, g], band(PB_ZI + 1),
                                 x8_pair(t, gs, 2, 0),
                                 start=False, stop=True, perf_mode=DR)

            def emit_zi_cvt(t, g, zp):
                gs = GS[g]
                nc.scalar.activation(ziall[:, t, 0, gs:gs + BG, 1:1 + W],
                                     zp[:, g], AF.Identity, bias=cols[:, 0:1])

            def emit_zi_dup(t):
                nc.gpsimd.tensor_copy(ziall[:, t, 1], ziall[:, t, 0])

            # ---- gates: 4 chunks (i, g, f, o) of 2 co each ----
            def emit_gates(t, g):
                gs = GS[g]
                chunks = []
                for ck in range(4):
                    gp = ppool.tile([H, 2, BG, W], F32, tag="g", bufs=5)
                    for cl in range(2):
                        co = ck * 2 + cl
                        mms = [(band(PB_IH(co)), zi_pair(t, gs, 0, 1, 1)),
                               (band(PB_IH(co) + 1), zi_pair(t, gs, 2, 2, 0))]
                        if t > 0:
                            for dx in range(3):
                                mms.append((band(PB_HH(co, dx)),
                                            h_pair(g, dx)))
                        for k, (w, rhs) in enumerate(mms):
                            nc.tensor.matmul(gp[:, cl], w, rhs, start=(k == 0),
                                             stop=(k == len(mms) - 1),
                                             perf_mode=DR)
                    chunks.append(gp)
                return chunks

            def emit_tg(t, g, chunks):
                tgs = []
                for ck, gp in enumerate(chunks):
                    tg = wpool.tile([H, 2, BG, W], BF16, tag=f"tg{ck}_{g}")
                    for (c0, n, bv) in act_runs[ck]:
                        nc.scalar.activation(tg[:, c0:c0 + n],
                                             gp[:, c0:c0 + n], AF.Tanh,
                                             scale=0.5, bias=bv)
                    tgs.append(tg)
                return tgs

            def emit_cellmath(t, g, tgs):
                gs = GS[g]
                ti, tgg, tf, to = tgs
                chs = ch2[:, :, gs:gs + BG, :]
                u2 = wpool.tile([H, 2, BG, W], BF16, tag=f"u2{g}")
                u1 = wpool.tile([H, 2, BG, W], BF16, tag=f"u1{g}")
                nc.vector.scalar_tensor_tensor(u2[:], ti[:], 1.0, tgg[:],
                                               ALU.add, ALU.mult)
                nc.vector.scalar_tensor_tensor(u1[:], tf[:], 1.0, chs,
                                               ALU.add, ALU.mult)
                nc.vector.scalar_tensor_tensor(chs, u1[:], 0.5, u2[:],
                                               ALU.mult, ALU.add)

            def emit_tail(t, g, tgs):
                gs = GS[g]
                to = tgs[3]
                tcn = wpool.tile([H, 2, BG, W], BF16, tag=f"tcn{g}")
                nc.scalar.activation(tcn[:], ch2[:, :, gs:gs + BG, :],
                                     AF.Tanh, scale=0.5)
                nc.vector.scalar_tensor_tensor(
                    hpair[:, g, :, :, 1:1 + W], to[:], 1.0, tcn[:],
                    ALU.add, ALU.mult)

            # ---- conv_out + logprob for channel t+1 (PE part) ----
            def emit_out(t, g):
                po = ppool.tile([H, 2, BG, W], F32, tag="o", bufs=2)
                for co in range(2):
                    for dx in range(3):
                        nc.tensor.matmul(po[:, co], band(PB_OUT(co, dx)),
                                         h_pair(g, dx), start=(dx == 0),
                                         stop=(dx == 2), perf_mode=DR)
                return po

            def emit_logprob(t, g, po):
                gs = GS[g]
                E = wpool.tile([H, BG, W], BF16, tag=f"E{g}")
                nc.scalar.activation(E[:], po[:, 1], AF.Exp, scale=-1.0,
                                     bias=cols[:, 1:2])
                d = wpool.tile([H, BG, W], BF16, tag=f"d{g}")
                nc.vector.scalar_tensor_tensor(
                    d[:], po[:, 0], cols[:, 4:5],
                    xbf[:, t + 1, gs:gs + BG, 1:1 + W], ALU.add, ALU.subtract)
                z = wpool.tile([H, BG, W], BF16, tag=f"z{g}")
                nc.gpsimd.tensor_tensor(z[:], d[:], E[:], ALU.mult)
                zj = wpool.tile([H, BG, W], BF16, tag=f"zj{g}")
                for im in range(BG):
                    nc.vector.scalar_tensor_tensor(
                        zj[:, im], z[:, im], 1.0, z[:, im], ALU.mult, ALU.mult,
                        accum_out=sqcols[:, gs + im, t:t + 1])
                nc.vector.tensor_reduce(lscols[:, gs:gs + BG, t:t + 1],
                                        po[:, 1], axis=mybir.AxisListType.X,
                                        op=ALU.add)

            # ---- prologue ----
            for t in range(min(2, TR)):
                zp = zi_alloc()
                emit_zi_mms(t, 0, zp)
                emit_zi_mms(t, 1, zp)
                emit_zi_cvt(t, 0, zp)
                emit_zi_cvt(t, 1, zp)
                emit_zi_dup(t)

            # channel 0 logprob: z0 = (x0 - b0) * exp(-b1)
            zjunk = wpool.tile([H, BL, W], BF16, tag="zjunk")
            for im in range(BL):
                nc.scalar.activation(
                    zjunk[:, im], xbf[:, 0, im, 1:1 + W], AF.Square,
                    scale=cols[:, 2:3], bias=cols[:, 3:4],
                    accum_out=sqcols[:, im, 15:16])

            # ---- recurrence (conv_out deferred one step on the PE) ----
            for t in range(TR):
                if t > 0:
                    pos = [emit_out(t - 1, g) for g in range(2)]
                    for g in range(2):
                        emit_logprob(t - 1, g, pos[g])
                chunks = []
                zp = zi_alloc() if t + 2 < TR else None
                for g in range(2):
                    chunks.append(emit_gates(t, g))
                    if zp is not None:
                        emit_zi_mms(t + 2, g, zp)
                tgs = [emit_tg(t, g, chunks[g]) for g in range(2)]
                for g in range(2):
                    emit_cellmath(t, g, tgs[g])
                if zp is not None:
                    for g in range(2):
                        emit_zi_cvt(t + 2, g, zp)
                for g in range(2):
                    emit_tail(t, g, tgs[g])
                if t + 2 < TR:
                    emit_zi_dup(t + 2)
            for g in range(2):
                po = emit_out(TR - 1, g)
                emit_logprob(TR - 1, g, po)

            # ---- final reduction ----
            s_sq = wpool.tile([H, BL, 1], F32, tag="ssq")
            s_ls = wpool.tile([H, BL, 1], F32, tag="sls")
            nc.vector.tensor_reduce(s_sq[:], sqcols[:],
                                    axis=mybir.AxisListType.X, op=ALU.add)
            nc.vector.tensor_reduce(s_ls[:], lscols[:],
                                    axis=mybir.AxisListType.X, op=ALU.add)
            comb = wpool.tile([H, BL], F32, tag="comb")
            nc.vector.scalar_tensor_tensor(comb[:], s_sq[:, :, 0], -0.5,
                                           s_ls[:, :, 0], ALU.mult,
                                           ALU.subtract)
            fps = ppool.tile([BL, 1], F32, tag="o", bufs=2)
            nc.tensor.matmul(fps[:], comb[:], ones[:], start=True, stop=True)
            osb = wpool.tile([BL, 1], F32, tag="osb")
            nc.vector.tensor_copy(osb[:], fps[:])
            nc.sync.dma_start(out_d[:], osb[:])

    nc.compile()
    return nc


def _get_program(act_runs):
    if act_runs not in _CACHED:
        _CACHED[act_runs] = _build_program(act_runs)
    return _CACHED[act_runs]


def kernel(x, Win, b_in, Wih, b_ih, Whh, b_hh, Wout, b_out):
    x = np.asarray(x, np.float32)
    Win = np.asarray(Win, np.float32)
    Wih = np.asarray(Wih, np.float32)
    Whh = np.asarray(Whh, np.float32)
    Wout = np.asarray(Wout, np.float32)
    b_in = np.asarray(b_in, np.float32)
    gb = (np.asarray(b_ih, np.float32) + np.asarray(b_hh, np.float32))
    b0, b1 = [float(v) for v in np.asarray(b_out, np.float32)]

    bands8, resid = _build_bands(Win, Wih, Whh, Wout, gb.astype(np.float64))

    # ACT bias runs per 2-co chunk: merge cos with equal residual bias
    def runs_for(ck):
        runs = []
        for cl in range(2):
            bv = float(resid[ck * 2 + cl])
            if abs(bv) < 1e-7:
                bv = 0.0
            if runs and runs[-1][2] == bv:
                runs[-1] = (runs[-1][0], runs[-1][1] + 1, bv)
            else:
                runs.append((cl, 1, bv))
        return tuple(runs)
    act_runs = tuple(runs_for(ck) for ck in range(4))

    cols = np.zeros((H, 16), np.float32)
    cols[:, 0] = float(b_in[0])
    cols[:, 1] = -b1
    cols[:, 2] = np.exp(-b1)
    cols[:, 3] = -b0 * np.exp(-b1)
    cols[:, 4] = b0

    bands_flat = np.ascontiguousarray(
        np.transpose(bands8, (2, 0, 1, 3))).view(np.uint8).reshape(H, -1)

    ones_u8 = np.full((H, T * BL * WP), 0x38, np.uint8)
    in_maps = []
    for k in range(NCORES):
        xs = x[k * BL:(k + 1) * BL]               # (BL, C, H, W)
        xpad = np.zeros((C, H, BL, WP), np.float32)
        xpad[:, :, :, 1:1 + W] = np.transpose(xs, (1, 2, 0, 3))
        xh = np.ascontiguousarray(np.transpose(xpad, (1, 0, 2, 3)))  # H,C,BL,WP
        x8 = xh[:, :T].astype(E4M3)               # H,T,BL,WP
        x8d = np.ascontiguousarray(np.stack([x8, x8], axis=2))  # H,T,2,BL,WP
        in_maps.append({
            "xbf": np.ascontiguousarray(xh.astype(ml_dtypes.bfloat16)
                                        ).reshape(H, -1),
            "x8": x8d.view(np.uint8).reshape(H, -1),
            "bands": bands_flat,
            "cols": cols,
            "ones8": ones_u8,
        })

    nc = _get_program(act_runs)
    global _last_in_maps, _last_nc
    _last_in_maps = in_maps
    _last_nc = nc
    res = run_bass_kernel_spmd(nc, in_maps, core_ids=list(range(NCORES)))

    const = -0.5 * LOG2PI * (H * W * C) - H * W * C * b1
    out = np.zeros((B,), np.float32)
    for k in range(NCORES):
        out[k * BL:(k + 1) * BL] = res.results[k]["out"].reshape(BL) + const
    return out
